# revision 1
# baseline (speedup 1.0000x reference)
"""IoU metric kernel for Trainium2 (Bass/Tile), 8-core data-parallel over batch.

Problem: input [16,21,512,512] f32 logits, target [16,21,512,512] f32 0/1 masks.
  pred = argmax_C(input); per-(b,c): inter = sum(target * onehot(pred)),
  gt = sum(target), pr = sum(onehot(pred)); present = any(target) = (gt > 0).
  scores[c] = (sum_b present*inter) / (sum_b present*(gt+pr) - inter_s + eps) * counts
Returns (scores[1:], counts[1:]).

Sharding: batch 16 -> 8 cores x 2 images. Each core computes per-image [C,3]
partials (inter, gt, pr); host does the trivial cross-batch combine.

Per-core kernel layout: image pixel plane [512,512] split into chunks of 128
h-rows: tile [128 part, 21 classes, 512 w]. Engines:
  DVE : 20-op running-max chain + 21 tensor_tensor(is_equal) ops producing
        the one-hot (bf16 - exact for 0/1).
  POOL: one big prod = oh * t multiply (bf16) - offloads DVE; runs
        concurrently with the pr/gt matmul pass (separate prod tile).
  PE  : per-class selector-matmuls (E_c.T @ rhs adds colsum into PSUM row c)
        accumulating inter/gt/pr into three PSUM banks across chunks.
  DMA : HWDGE (nc.sync) big coalesced loads; target pre-cast to bf16 on the
        host (exact for 0/1 masks, halves target HBM traffic).
Known-good/bad: tensor_tensor_reduce and SWDGE cast-DMA were tried and
rejected (TTR crashes real HW despite passing CoreSim; SWDGE descriptor
generation on Q7 serializes with the POOL multiply).
"""

import os
import threading
from contextlib import ExitStack

import numpy as np

import concourse.bacc as bacc
import concourse.bass as bass
import concourse.mybir as mybir
import concourse.tile as tile
from concourse.alu_op_type import AluOpType
from concourse.bass_utils import run_bass_kernel_spmd

F32 = mybir.dt.float32
BF16 = mybir.dt.bfloat16

B, C, H, W = 16, 21, 512, 512
NCORES = 8
BPC = B // NCORES  # images per core
P = 128

# Tunables
USE_POOL_MUL = os.environ.get("IOU_POOL_MUL", "1") == "1"
T_BF16 = os.environ.get("IOU_T_BF16", "1") == "1"
INPLACE_MUL = os.environ.get("IOU_INPLACE_MUL", "0") == "1"


def build_kernel_ir(nc, bpc=BPC, n_classes=C, h=H, w=W):
    """Emit the Tile IR for one core's shard [bpc, n_classes, h, w]."""
    f = w  # free dim per chunk = image width
    chunks = h // P  # chunks per image (h rows of 128)

    t_dram_dt = BF16 if T_BF16 else F32
    inp = nc.dram_tensor("input", [bpc, n_classes, h, w], F32, kind="ExternalInput")
    tgt = nc.dram_tensor("target", [bpc, n_classes, h, w], t_dram_dt, kind="ExternalInput")
    stats = nc.dram_tensor("stats", [bpc, n_classes, 4], F32, kind="ExternalOutput")

    # [b, c, (j p), w] -> [b, j, p, c, w]
    inp_r = inp.ap().rearrange("b c (j p) w -> b j p c w", p=P)
    tgt_r = tgt.ap().rearrange("b c (j p) w -> b j p c w", p=P)
    stats_ap = stats.ap()

    t_dt = BF16 if T_BF16 else F32

    with tile.TileContext(nc) as tc, ExitStack() as ctx:
        data_pool = ctx.enter_context(tc.tile_pool(name="data", bufs=2))
        acc_pool = ctx.enter_context(tc.tile_pool(name="acc", bufs=1))
        out_pool = ctx.enter_context(tc.tile_pool(name="outp", bufs=1))
        psum_pool = ctx.enter_context(tc.tile_pool(name="psum", bufs=1, space="PSUM"))

        # Per-class selector weights: E[:, c, :] is [128, C] with column c all
        # ones -> matmul(E_c.T @ rhs) adds colsum(rhs) into PSUM row c only.
        sel_dt = BF16 if T_BF16 else F32
        sel = acc_pool.tile([P, n_classes, n_classes], sel_dt, tag="sel")
        nc.vector.memset(sel, 0.0)
        for c in range(n_classes):
            nc.vector.memset(sel[:, c, c : c + 1], 1.0)

        for img in range(bpc):
            psum_inter = psum_pool.tile([n_classes, f], F32, tag=f"pi{img}")
            psum_gt = psum_pool.tile([n_classes, f], F32, tag=f"pg{img}")
            psum_pr = psum_pool.tile([n_classes, f], F32, tag=f"pp{img}")

            for j in range(chunks):
                xb = data_pool.tile([P, n_classes, f], F32, tag="xb")
                nc.sync.dma_start(out=xb[:], in_=inp_r[img, j])
                tb = data_pool.tile([P, n_classes, f], t_dt, tag="tb")
                nc.sync.dma_start(out=tb[:], in_=tgt_r[img, j])

                # running max over classes, split DVE / POOL as two subtrees
                pm = int(os.environ.get("IOU_POOL_MAX", "0"))
                split = n_classes - pm if pm >= 2 else n_classes
                m = data_pool.tile([P, f], F32, tag="m")
                nc.vector.tensor_max(m[:], xb[:, 0, :], xb[:, 1, :])
                for c in range(2, split):
                    nc.vector.tensor_max(m[:], m[:], xb[:, c, :])
                if split < n_classes:
                    mp = data_pool.tile([P, f], F32, tag="mp")
                    nc.gpsimd.tensor_max(mp[:], xb[:, split, :], xb[:, split + 1, :])
                    for c in range(split + 2, n_classes):
                        nc.gpsimd.tensor_max(mp[:], mp[:], xb[:, c, :])
                    nc.vector.tensor_max(m[:], m[:], mp[:])

                # one-hot via is_equal vs the max
                oh_dt = BF16 if T_BF16 else F32
                oh = data_pool.tile(
                    [P, n_classes, f], oh_dt, tag="oh",
                    bufs=(2 if INPLACE_MUL else 1),
                )
                for c in range(n_classes):
                    nc.vector.tensor_tensor(
                        oh[:, c, :], xb[:, c, :], m[:], AluOpType.is_equal
                    )

                # pr matmuls must read oh before the (possibly in-place) mul
                for c in range(n_classes):
                    first = j == 0 and c == 0
                    last = j == chunks - 1 and c == n_classes - 1
                    nc.tensor.matmul(
                        psum_pr[:, :], sel[:, c, :], oh[:, c, :],
                        start=first, stop=last,
                    )
                    nc.tensor.matmul(
                        psum_gt[:, :], sel[:, c, :], tb[:, c, :],
                        start=first, stop=last,
                    )

                # prod = oh * t
                if INPLACE_MUL:
                    prod = oh
                else:
                    prod = data_pool.tile([P, n_classes, f], oh_dt, tag="prod", bufs=1)
                mul_eng = nc.gpsimd if USE_POOL_MUL else nc.vector
                if os.environ.get("IOU_SPLIT_MUL", "1") == "1":
                    half = n_classes // 2
                    mul_eng.tensor_mul(
                        prod[:, :half, :], oh[:, :half, :], tb[:, :half, :]
                    )
                    mul_eng.tensor_mul(
                        prod[:, half:, :], oh[:, half:, :], tb[:, half:, :]
                    )
                else:
                    mul_eng.tensor_mul(prod[:], oh[:], tb[:])

                for c in range(n_classes):
                    first = j == 0 and c == 0
                    last = j == chunks - 1 and c == n_classes - 1
                    nc.tensor.matmul(
                        psum_inter[:, :], sel[:, c, :], prod[:, c, :],
                        start=first, stop=last,
                    )

            # finalize image: [C,f] psum -> [C,1]; pr partition-reduce via PE
            res = out_pool.tile([n_classes, 4], F32, tag=f"res{img}")
            nc.vector.tensor_reduce(
                out=res[:, 0:1], in_=psum_inter[:], axis=mybir.AxisListType.X,
                op=AluOpType.add,
            )
            nc.vector.tensor_reduce(
                out=res[:, 1:2], in_=psum_gt[:], axis=mybir.AxisListType.X,
                op=AluOpType.add,
            )
            nc.vector.tensor_reduce(
                out=res[:, 2:3], in_=psum_pr[:], axis=mybir.AxisListType.X,
                op=AluOpType.add,
            )
            nc.vector.memset(res[:, 3:4], 0.0)
            nc.sync.dma_start(out=stats_ap[img], in_=res[:])

    return nc


_BUILD_LOCK = threading.Lock()
_NC_CACHE = {}


def get_compiled_nc(key="full"):
    with _BUILD_LOCK:
        if key not in _NC_CACHE:
            nc = bacc.Bacc("TRN2", target_bir_lowering=False, debug=False)
            build_kernel_ir(nc)
            nc.compile()
            _NC_CACHE[key] = nc
        return _NC_CACHE[key]


def combine_stats(stats_all):
    """stats_all: [B, C, >=3] per-image partials -> (scores[1:], counts[1:])."""
    stats_all = np.asarray(stats_all, dtype=np.float64)
    inter_bc = stats_all[..., 0]
    gt_bc = stats_all[..., 1]
    pr_bc = stats_all[..., 2]
    present = (gt_bc > 0).astype(np.float64)
    inter_s = (present * inter_bc).sum(0)
    union_s = (present * (gt_bc + pr_bc)).sum(0) - inter_s + 1e-7
    counts = present.sum(0)
    scores = (inter_s / union_s) * counts
    return (
        scores[1:].astype(np.float32),
        counts[1:].astype(np.float32),
    )


def kernel(input, target):
    import ml_dtypes

    inp = np.ascontiguousarray(np.asarray(input, dtype=np.float32))
    tgt = np.ascontiguousarray(np.asarray(target, dtype=np.float32))
    assert inp.shape == (B, C, H, W), inp.shape
    if T_BF16:
        # 0/1 masks are exact in bf16; halves target HBM traffic on-device
        tgt = tgt.astype(ml_dtypes.bfloat16)

    nc = get_compiled_nc()
    in_maps = [
        {
            "input": inp[i * BPC : (i + 1) * BPC],
            "target": tgt[i * BPC : (i + 1) * BPC],
        }
        for i in range(NCORES)
    ]
    res = run_bass_kernel_spmd(nc, in_maps, core_ids=list(range(NCORES)))
    stats_all = np.concatenate([r["stats"] for r in res.results], axis=0)  # [B,C,4]
    return combine_stats(stats_all)


if __name__ == "__main__":
    rng = np.random.default_rng(0)
    x = rng.standard_normal((B, C, H, W), dtype=np.float32)
    t = (rng.random((B, C, H, W)) < 0.05).astype(np.float32)
    s, c = kernel(input=x, target=t)
    print("scores:", s)
    print("counts:", c)



# revision 7
# speedup vs baseline: 2.1698x; 2.1698x over previous
"""IoU metric kernel for Trainium2 (Bass/Tile), 8-core data-parallel over batch.

Problem: input [16,21,512,512] f32 logits, target [16,21,512,512] f32 0/1 masks.
  pred = argmax_C(input); per-(b,c): inter = sum(target * onehot(pred)),
  gt = sum(target), pr = sum(onehot(pred)); present = (gt > 0).
  scores[c] = (sum_b present*inter) / (sum_b present*(gt+pr) - inter_s + eps) * counts
Returns (scores[1:], counts[1:]).

Sharding: batch 16 -> 8 cores x 2 images; host combines per-image partials.

v5 design (DMA-bound ~14.9us per [128,21,512] chunk; 8 chunks/core):
  Host casts input to fp16 (fp16 argmax ties cost rel-err ~1e-3, well under the
  2e-2 gate) and target channels 1..20 to fp16 (class 0 is never scored).
  Hardware constraints discovered on this toolchain (walrus engine checks +
  scheduler behavior):
    - Pool/Q7 TensorTensor supports ONLY Multiply/Add (no max, no is_equal),
      so the max tree and all is_equal must run on DVE.
    - Any DVE<->Pool dependency cycle is serialized by the scheduler into a
      ~25us/step round-robin; keep cross-engine edges one-way in time.
    - PE drops to mid-pstate (2x slower) whenever its stream has gaps; feed it
      one contiguous block of matmuls per step, delayed until all inputs are
      complete.
  Per chunk t:
    SP  : input DMA split xbA (classes 0..9) / xbB (10..20) then tb -- lets the
          DVE tree start ~4us earlier in the fill; tb lands last.
    DVE : max tree (7 instrs: paired stage0 over xbA/xbB halves, then halving),
          is_equal x20 (Pool-mul classes first), prod-mul for 7 classes
          (all 20 on the final chunk to shorten the tail).
    Pool: prod-mul for 13 classes (its only legal op here; ~13.3us, lags one
          window -- one-way DVE->Pool edge only).
    ACT : gt sums via activation(Copy, accum_out) for K_ACT classes + PSUM
          X-reduce drains; drain DMAs go out on ACT's queue so they never
          head-block the SP load queue.
    PE  : one contiguous block per step of thin selector matmuls (stationary
          [128,20] one-hot column): pr_{t-1}, interA_{t-1}, interB_{t-2},
          gt_{t-1} -- every input complete before the block starts.
  Known-bad (prior sessions): tensor_tensor_reduce crashes real HW; SWDGE
  cast-DMA serializes on Q7; Pool max/is_equal fail walrus engine checks.
"""

import threading
from contextlib import ExitStack

import numpy as np

import concourse.bacc as bacc
import concourse.mybir as mybir
import concourse.tile as tile
from concourse.alu_op_type import AluOpType
from concourse.bass_utils import run_bass_kernel_spmd

F32 = mybir.dt.float32
F16 = mybir.dt.float16

B, C, H, W = 16, 21, 512, 512
CT = C - 1  # scored classes (1..20)
NCORES = 8
BPC = B // NCORES  # images per core
P = 128

K_ACT = 10   # gt classes summed on ACT; classes K_ACT..19 go to PE
K_DVE = 7    # prod-mul classes on DVE (0..K_DVE-1); the rest go to Pool


def build_kernel_ir(nc, bpc=BPC, h=H, w=W):
    """Emit the Tile IR for one core's shard: input [bpc,21,h,w] fp16,
    target [bpc,20,h,w] fp16 -> stats [bpc,20,4] f32 + gtacc [bpc,128,4,K_ACT]."""
    chunks = h // P  # h-chunks per image
    nsteps = bpc * chunks

    inp = nc.dram_tensor("input", [bpc, C, h, w], F16, kind="ExternalInput")
    tgt = nc.dram_tensor("target", [bpc, CT, h, w], F16, kind="ExternalInput")
    stats = nc.dram_tensor("stats", [bpc, CT, 4], F32, kind="ExternalOutput")
    gtacc = nc.dram_tensor("gtacc", [bpc, P, chunks, K_ACT], F32,
                           kind="ExternalOutput")

    inp_r = inp.ap().rearrange("b c (j p) w -> b j p c w", p=P)
    tgt_r = tgt.ap().rearrange("b c (j p) w -> b j p c w", p=P)
    stats_ap = stats.ap()
    gtacc_ap = gtacc.ap()

    Copy = mybir.ActivationFunctionType.Copy

    with tile.TileContext(nc) as tc, ExitStack() as ctx:
        data_pool = ctx.enter_context(tc.tile_pool(name="data", bufs=2))
        scr_pool = ctx.enter_context(tc.tile_pool(name="scr", bufs=1))
        acc_pool = ctx.enter_context(tc.tile_pool(name="acc", bufs=1))
        psum_pool = ctx.enter_context(tc.tile_pool(name="psum", bufs=1, space="PSUM"))

        # Selector weights: sel[:, c, :] is [128, CT] with column c all ones.
        sel = acc_pool.tile([P, CT, CT], F16, tag="sel")
        nc.vector.memset(sel, 0.0)
        for c in range(CT):
            nc.vector.memset(sel[:, c, c : c + 1], 1.0)

        junk32 = acc_pool.tile([P, w], F32, tag="junk32")  # ACT scratch out

        gt_accs = [
            acc_pool.tile([P, chunks, K_ACT], F32, tag=f"gta{i}", name=f"gta{i}")
            for i in range(bpc)
        ]
        ps_pr = [psum_pool.tile([CT, w], F32, tag=f"pr{i}", name=f"pspr{i}")
                 for i in range(bpc)]
        ps_in = [psum_pool.tile([CT, w], F32, tag=f"in{i}", name=f"psin{i}")
                 for i in range(bpc)]
        ps_gt = [psum_pool.tile([CT, w], F32, tag=f"gt{i}", name=f"psgt{i}")
                 for i in range(bpc)]

        # Saved per-step tile refs for the delayed PE blocks.
        saved = {}  # t -> dict(img, j, oh, prod, tb, kdve)

        def emit_pr_interA_gt(s):
            """PE block part 1 for source step s: pr x20, interA, gt."""
            st = saved[s]
            img, j, kdve = st["img"], st["j"], st["kdve"]
            first, last = j == 0, j == chunks - 1
            for c in range(CT):
                nc.tensor.matmul(
                    ps_pr[img][:, :], sel[:, c, :], st["oh"][:, c, :],
                    start=(first and c == 0), stop=(last and c == CT - 1),
                )
            for c in range(kdve):
                nc.tensor.matmul(
                    ps_in[img][:, :], sel[:, c, :], st["prod"][:, c, :],
                    start=(first and c == 0),
                    stop=(last and kdve == CT and c == CT - 1),
                )
            for c in range(K_ACT, CT):
                nc.tensor.matmul(
                    ps_gt[img][:, :], sel[:, c, :], st["tb"][:, c, :],
                    start=(first and c == K_ACT), stop=(last and c == CT - 1),
                )

        def emit_interB(s):
            """PE block part 2 for source step s: inter for Pool-mul classes."""
            st = saved[s]
            img, j, kdve = st["img"], st["j"], st["kdve"]
            last = j == chunks - 1
            for c in range(kdve, CT):
                nc.tensor.matmul(
                    ps_in[img][:, :], sel[:, c, :], st["prod"][:, c, :],
                    start=False, stop=(last and c == CT - 1),
                )

        def drain_image(img):
            # ACT X-reduces of the image's PSUM banks (DVE is busier).
            res = acc_pool.tile([CT, 4], F32, tag=f"res{img}", name=f"res{img}")
            nc.scalar.activation(junk32[0:CT, :], ps_pr[img][:], Copy,
                                 accum_out=res[:, 0:1])
            nc.scalar.activation(junk32[0:CT, :], ps_in[img][:], Copy,
                                 accum_out=res[:, 1:2])
            if K_ACT < CT:
                nc.scalar.activation(junk32[0:CT, :], ps_gt[img][:], Copy,
                                     accum_out=res[:, 2:3])
            nc.vector.memset(res[:, 3:4], 0.0)
            nc.scalar.dma_start(out=stats_ap[img], in_=res[:])
            nc.scalar.dma_start(out=gtacc_ap[img], in_=gt_accs[img][:])

        for t in range(nsteps):
            img, j = divmod(t, chunks)
            # Final chunk: DVE muls everything so the tail skips Pool's 13us.
            kdve = CT if t == nsteps - 1 else K_DVE

            xb = data_pool.tile([P, C, w], F16, tag="xb")
            nc.sync.dma_start(out=xb[:, 0:10, :], in_=inp_r[img, j, :, 0:10])
            nc.sync.dma_start(out=xb[:, 10:21, :], in_=inp_r[img, j, :, 10:21])
            tb = data_pool.tile([P, CT, w], F16, tag="tb")
            nc.sync.dma_start(out=tb[:], in_=tgt_r[img, j])

            # ---- DVE: max tree (stage0 split over the two DMA halves) ----
            m10 = scr_pool.tile([P, 10, w], F16, tag="m10")
            nc.vector.tensor_tensor(
                m10[:, 0:5, :], xb[:, 0:9:2, :], xb[:, 1:10:2, :], AluOpType.max
            )
            nc.vector.tensor_tensor(
                m10[:, 5:10, :], xb[:, 10:19:2, :], xb[:, 11:20:2, :],
                AluOpType.max,
            )
            m5 = scr_pool.tile([P, 5, w], F16, tag="m5")
            nc.vector.tensor_tensor(
                m5[:], m10[:, 0:5, :], m10[:, 5:10, :], AluOpType.max
            )
            m2 = scr_pool.tile([P, 2, w], F16, tag="m2")
            nc.vector.tensor_tensor(
                m2[:], m5[:, 0:2, :], m5[:, 2:4, :], AluOpType.max
            )
            m1 = scr_pool.tile([P, w], F16, tag="m1")
            nc.vector.tensor_tensor(m1[:], m2[:, 0, :], m2[:, 1, :], AluOpType.max)
            m1b = scr_pool.tile([P, w], F16, tag="m1b")
            nc.vector.tensor_tensor(m1b[:], m1[:], m5[:, 4, :], AluOpType.max)
            m = data_pool.tile([P, w], F16, tag="m")
            nc.vector.tensor_tensor(m[:], m1b[:], xb[:, 20, :], AluOpType.max)

            oh = data_pool.tile([P, CT, w], F16, tag="oh")
            prod = data_pool.tile([P, CT, w], F16, tag="prod")

            # ---- DVE: is_equal x20 (Pool's mul classes first) + its muls ----
            for c in list(range(kdve, CT)) + list(range(kdve)):
                nc.vector.tensor_tensor(
                    oh[:, c, :], xb[:, c + 1, :], m[:], AluOpType.is_equal
                )
            nc.vector.tensor_tensor(
                prod[:, :kdve, :], oh[:, :kdve, :], tb[:, :kdve, :],
                AluOpType.mult,
            )

            # ---- Pool: prod-mul for its classes (lags ~one window) ----
            if kdve < CT:
                nc.gpsimd.tensor_tensor(
                    prod[:, kdve:, :], oh[:, kdve:, :], tb[:, kdve:, :],
                    AluOpType.mult,
                )

            # ---- ACT: gt sums for K_ACT classes ----
            for c in range(K_ACT):
                nc.scalar.activation(
                    junk32[:], tb[:, c, :], Copy,
                    accum_out=gt_accs[img][:, j, c : c + 1],
                )

            saved[t] = dict(img=img, j=j, oh=oh, prod=prod, tb=tb, kdve=kdve)

            # ---- PE: contiguous delayed block ----
            if t >= 1:
                emit_pr_interA_gt(t - 1)
            if t >= 2 and saved[t - 2]["kdve"] < CT:
                emit_interB(t - 2)
            if t - 2 in saved:
                del saved[t - 2]

            # img0 drain: its inter group stops at interB(3), emitted at t=5.
            if t == chunks + 1 and bpc > 1:
                drain_image(0)

        # Flush: last two steps' PE blocks (interB first so the final image's
        # inter group still ends on interA(nsteps-1)'s stop), then drain.
        if saved[nsteps - 2]["kdve"] < CT:
            emit_interB(nsteps - 2)
        emit_pr_interA_gt(nsteps - 1)
        drain_image(bpc - 1)

    return nc


_BUILD_LOCK = threading.Lock()
_NC_CACHE = {}


def get_compiled_nc(key="full"):
    with _BUILD_LOCK:
        if key not in _NC_CACHE:
            nc = bacc.Bacc("TRN2", target_bir_lowering=False, debug=False)
            build_kernel_ir(nc)
            nc.compile()
            _NC_CACHE[key] = nc
        return _NC_CACHE[key]


def make_in_maps(input, target):
    """Full f32 inputs -> per-core input dicts (fp16, class 0 dropped)."""
    inp = np.asarray(input, dtype=np.float32)
    tgt = np.asarray(target, dtype=np.float32)
    assert inp.shape == (B, C, H, W), inp.shape
    inp16 = inp.astype(np.float16)
    tgt16 = np.ascontiguousarray(tgt[:, 1:, :, :]).astype(np.float16)
    return [
        {
            "input": np.ascontiguousarray(inp16[i * BPC : (i + 1) * BPC]),
            "target": np.ascontiguousarray(tgt16[i * BPC : (i + 1) * BPC]),
        }
        for i in range(NCORES)
    ]


def combine_stats(stats_all, gtacc_all):
    """stats_all [B,CT,4], gtacc_all [B,P,chunks,K_ACT] -> (scores, counts)."""
    stats_all = np.asarray(stats_all, dtype=np.float64)
    pr_bc = stats_all[..., 0]
    inter_bc = stats_all[..., 1]
    gt_bc = stats_all[..., 2].copy()
    gt_act = np.asarray(gtacc_all, dtype=np.float64).sum(axis=(1, 2))  # [B,K_ACT]
    gt_bc[:, :K_ACT] = gt_act
    present = (gt_bc > 0).astype(np.float64)
    inter_s = (present * inter_bc).sum(0)
    union_s = (present * (gt_bc + pr_bc)).sum(0) - inter_s + 1e-7
    counts = present.sum(0)
    scores = (inter_s / union_s) * counts
    return scores.astype(np.float32), counts.astype(np.float32)


def kernel(input, target):
    nc = get_compiled_nc()
    in_maps = make_in_maps(input, target)
    res = run_bass_kernel_spmd(nc, in_maps, core_ids=list(range(NCORES)))
    stats_all = np.concatenate([r["stats"] for r in res.results], axis=0)
    gtacc_all = np.concatenate([r["gtacc"] for r in res.results], axis=0)
    return combine_stats(stats_all, gtacc_all)


if __name__ == "__main__":
    rng = np.random.default_rng(0)
    x = rng.standard_normal((B, C, H, W), dtype=np.float32)
    t = (rng.random((B, C, H, W)) < 0.05).astype(np.float32)
    s, c = kernel(input=x, target=t)
    print("scores:", s)
    print("counts:", c)


# revision 11
# speedup vs baseline: 2.3351x; 1.0762x over previous
"""IoU metric kernel for Trainium2 (Bass/Tile), 8-core data-parallel over batch.

Problem: input [16,21,512,512] f32 logits, target [16,21,512,512] f32 0/1 masks.
  pred = argmax_C(input); per-(b,c): inter = sum(target * onehot(pred)),
  gt = sum(target), pr = sum(onehot(pred)); present = (gt > 0).
  scores[c] = (sum_b present*inter) / (sum_b present*(gt+pr) - inter_s + eps) * counts
Returns (scores[1:], counts[1:]).

Sharding: batch 16 -> 8 cores x 2 images; host combines per-image partials.

v7 design (~12.6us DMA / ~13.3us compute per [128,21,512] chunk; 8 chunks):
  Host casts input to fp16 (fp16 argmax ties cost rel-err ~1e-3, well under the
  2e-2 gate). Target drops class 0 (never scored); channels for classes 1..7
  ship as fp16 (consumed by DVE muls, which need 2-byte dtypes for the DVE 2x
  mode) and classes 8..20 as fp8e4 (exact for 0/1 masks; consumed by Pool,
  whose Q7 cost is dtype-flat, by ACT, and by PE -- none penalized by fp8).
  Hardware constraints discovered on this toolchain:
    - Pool/Q7 TensorTensor supports ONLY Multiply/Add (no max, no is_equal),
      so the max tree and all is_equal must run on DVE.
    - Any DVE<->Pool dependency cycle is serialized by the scheduler into a
      round-robin; keep cross-engine edges one-way in time.
    - PE drops to mid-pstate (2x slower) whenever its stream has gaps; feed it
      one contiguous block of matmuls per step, delayed until inputs are done.
    - Consumer semaphore waits get coalesced per contiguous emission run, so
      interleave PE consumers with their producers' batch boundaries.
  Per chunk t:
    SP  : loads xbA (classes 0..9), xbB (10..20), tb16, tb8.
    DVE : max tree (7 instrs), broadcast is_equal, prod-mul for the 7 fp16
          classes (plus 7 fp8 classes on the final chunk to shorten the tail).
    Pool: prod-mul for the 13 fp8 classes (~13.3us, lags one window).
    ACT : gt sums via activation(Copy, accum_out) for K_ACT classes + PSUM
          X-reduce drains; drain/output DMAs ride ACT's queue.
    PE  : one contiguous delayed block of thin selector matmuls (stationary
          [128,20] one-hot column): pr_{t-1}, interA_{t-1}, interB_{t-2},
          gt_{t-1}.
  Known-bad (prior sessions): tensor_tensor_reduce crashes real HW; SWDGE
  cast-DMA serializes on Q7; Pool max/is_equal fail walrus engine checks.
"""

import threading
from contextlib import ExitStack

import numpy as np

import concourse.bacc as bacc
import concourse.mybir as mybir
import concourse.tile as tile
from concourse.alu_op_type import AluOpType
from concourse.bass_utils import run_bass_kernel_spmd

F32 = mybir.dt.float32
F16 = mybir.dt.float16
F8 = mybir.dt.float8e4

B, C, H, W = 16, 21, 512, 512
CT = C - 1   # scored classes (1..20)
NCORES = 8
BPC = B // NCORES  # images per core
P = 128

K_ACT = 10   # gt classes summed on ACT; classes K_ACT..19 go to PE
N16 = 7      # target classes kept fp16 (DVE-mul classes 0..6)
N8 = CT - N16  # fp8 target classes (Pool-mul classes 7..19)
K_LAST = 14  # last chunk: DVE muls classes 0..13, Pool only 14..19


def build_kernel_ir(nc, bpc=BPC, h=H, w=W):
    """Emit the Tile IR for one core's shard: input [bpc,21,h,w] fp16,
    target [bpc,7,h,w] fp16 + target8 [bpc,13,h,w] fp8e4
    -> stats [bpc,20,4] f32 + gtacc [bpc,128,4,K_ACT] f32."""
    chunks = h // P  # h-chunks per image
    nsteps = bpc * chunks

    inp = nc.dram_tensor("input", [bpc, C, h, w], F16, kind="ExternalInput")
    tgt16 = nc.dram_tensor("target", [bpc, N16, h, w], F16, kind="ExternalInput")
    tgt8 = nc.dram_tensor("target8", [bpc, N8, h, w], F8, kind="ExternalInput")
    stats = nc.dram_tensor("stats", [bpc, CT, 4], F32, kind="ExternalOutput")
    gtacc = nc.dram_tensor("gtacc", [bpc, P, chunks, K_ACT], F32,
                           kind="ExternalOutput")

    inp_r = inp.ap().rearrange("b c (j p) w -> b j p c w", p=P)
    tgt16_r = tgt16.ap().rearrange("b c (j p) w -> b j p c w", p=P)
    tgt8_r = tgt8.ap().rearrange("b c (j p) w -> b j p c w", p=P)
    stats_ap = stats.ap()
    gtacc_ap = gtacc.ap()

    Copy = mybir.ActivationFunctionType.Copy

    with tile.TileContext(nc) as tc, ExitStack() as ctx:
        data_pool = ctx.enter_context(tc.tile_pool(name="data", bufs=2))
        scr_pool = ctx.enter_context(tc.tile_pool(name="scr", bufs=1))
        acc_pool = ctx.enter_context(tc.tile_pool(name="acc", bufs=1))
        psum_pool = ctx.enter_context(tc.tile_pool(name="psum", bufs=1, space="PSUM"))

        # Selector weights: sel[:, c, :] is [128, CT] with column c all ones.
        sel = acc_pool.tile([P, CT, CT], F16, tag="sel")
        nc.vector.memset(sel, 0.0)
        for c in range(CT):
            nc.vector.memset(sel[:, c, c : c + 1], 1.0)
        sel8 = acc_pool.tile([P, CT, CT], F8, tag="sel8")  # for fp8 moving
        nc.vector.memset(sel8, 0.0)
        for c in range(CT):
            nc.vector.memset(sel8[:, c, c : c + 1], 1.0)

        junk32 = acc_pool.tile([P, w], F32, tag="junk32")  # ACT scratch out

        gt_accs = [
            acc_pool.tile([P, chunks, K_ACT], F32, tag=f"gta{i}", name=f"gta{i}")
            for i in range(bpc)
        ]
        ps_pr = [psum_pool.tile([CT, w], F32, tag=f"pr{i}", name=f"pspr{i}")
                 for i in range(bpc)]
        ps_in = [psum_pool.tile([CT, w], F32, tag=f"in{i}", name=f"psin{i}")
                 for i in range(bpc)]
        ps_gt = [psum_pool.tile([CT, w], F32, tag=f"gt{i}", name=f"psgt{i}")
                 for i in range(bpc)]

        # Saved per-step tile refs for the delayed PE blocks.
        saved = {}  # t -> dict(img, j, oh, prod, tb8, kdve)

        def emit_pr_interA_gt(s):
            """PE block part 1 for source step s: pr x20, interA, gt."""
            st = saved[s]
            img, j, kdve = st["img"], st["j"], st["kdve"]
            first, last = j == 0, j == chunks - 1
            for c in range(CT):
                nc.tensor.matmul(
                    ps_pr[img][:, :], sel[:, c, :], st["oh"][:, c, :],
                    start=(first and c == 0), stop=(last and c == CT - 1),
                )
            for c in range(kdve):
                nc.tensor.matmul(
                    ps_in[img][:, :], sel[:, c, :], st["prod"][:, c, :],
                    start=(first and c == 0), stop=False,
                )
            for c in range(K_ACT, CT):
                nc.tensor.matmul(
                    ps_gt[img][:, :], sel8[:, c, :], st["tb8"][:, c - N16, :],
                    start=(first and c == K_ACT), stop=(last and c == CT - 1),
                )

        def emit_interB(s, stop_at_end):
            """PE block part 2 for source step s: inter for Pool-mul classes."""
            st = saved[s]
            img, kdve = st["img"], st["kdve"]
            for c in range(kdve, CT):
                nc.tensor.matmul(
                    ps_in[img][:, :], sel[:, c, :], st["prod"][:, c, :],
                    start=False, stop=(stop_at_end and c == CT - 1),
                )

        def drain_image(img):
            # ACT X-reduces of the image's PSUM banks (DVE is busier).
            # gtacc DMA first (no PSUM dep); inter drain last (gated latest).
            res = acc_pool.tile([CT, 4], F32, tag=f"res{img}", name=f"res{img}")
            nc.scalar.dma_start(out=gtacc_ap[img], in_=gt_accs[img][:])
            nc.scalar.activation(junk32[0:CT, :], ps_pr[img][:], Copy,
                                 accum_out=res[:, 0:1])
            if K_ACT < CT:
                nc.scalar.activation(junk32[0:CT, :], ps_gt[img][:], Copy,
                                     accum_out=res[:, 2:3])
            nc.scalar.activation(junk32[0:CT, :], ps_in[img][:], Copy,
                                 accum_out=res[:, 1:2])
            nc.scalar.dma_start(out=stats_ap[img], in_=res[:])

        for t in range(nsteps):
            img, j = divmod(t, chunks)
            # Final chunk: DVE takes most muls so the tail skips Pool's 13us.
            kdve = K_LAST if t == nsteps - 1 else N16

            xb = data_pool.tile([P, C, w], F16, tag="xb")
            nc.sync.dma_start(out=xb[:, 0:10, :], in_=inp_r[img, j, :, 0:10])
            nc.sync.dma_start(out=xb[:, 10:21, :], in_=inp_r[img, j, :, 10:21])
            tb16 = data_pool.tile([P, N16, w], F16, tag="tb16")
            nc.sync.dma_start(out=tb16[:], in_=tgt16_r[img, j])
            tb8 = data_pool.tile([P, N8, w], F8, tag="tb8")
            nc.sync.dma_start(out=tb8[:], in_=tgt8_r[img, j])

            # ---- DVE: max tree (stage0 split over the two DMA halves) ----
            m10 = scr_pool.tile([P, 10, w], F16, tag="m10")
            nc.vector.tensor_tensor(
                m10[:, 0:5, :], xb[:, 0:9:2, :], xb[:, 1:10:2, :], AluOpType.max
            )
            nc.vector.tensor_tensor(
                m10[:, 5:10, :], xb[:, 10:19:2, :], xb[:, 11:20:2, :],
                AluOpType.max,
            )
            m5 = scr_pool.tile([P, 5, w], F16, tag="m5")
            nc.vector.tensor_tensor(
                m5[:], m10[:, 0:5, :], m10[:, 5:10, :], AluOpType.max
            )
            m2 = scr_pool.tile([P, 2, w], F16, tag="m2")
            nc.vector.tensor_tensor(
                m2[:], m5[:, 0:2, :], m5[:, 2:4, :], AluOpType.max
            )
            m1 = scr_pool.tile([P, w], F16, tag="m1")
            nc.vector.tensor_tensor(m1[:], m2[:, 0, :], m2[:, 1, :], AluOpType.max)
            m1b = scr_pool.tile([P, w], F16, tag="m1b")
            nc.vector.tensor_tensor(m1b[:], m1[:], m5[:, 4, :], AluOpType.max)
            m = data_pool.tile([P, w], F16, tag="m")
            nc.vector.tensor_tensor(m[:], m1b[:], xb[:, 20, :], AluOpType.max)

            oh = data_pool.tile([P, CT, w], F16, tag="oh")
            prod = data_pool.tile([P, CT, w], F16, tag="prod")

            if kdve == N16:
                # ---- DVE: broadcast is_equal + fp16 muls ----
                mb = m[:].unsqueeze(1).broadcast_to([P, CT, w])
                nc.vector.tensor_tensor(
                    oh[:], xb[:, 1:21, :], mb, AluOpType.is_equal
                )
                nc.vector.tensor_tensor(
                    prod[:, :N16, :], oh[:, :N16, :], tb16[:], AluOpType.mult
                )
                # ---- Pool: fp8 muls (lags ~one window) ----
                nc.gpsimd.tensor_tensor(
                    prod[:, N16:, :], oh[:, N16:, :], tb8[:], AluOpType.mult
                )
            else:
                # Final chunk: iseq halves + mul batches so PE can track.
                mb10 = m[:].unsqueeze(1).broadcast_to([P, 10, w])
                nc.vector.tensor_tensor(
                    oh[:, 0:10, :], xb[:, 1:11, :], mb10, AluOpType.is_equal
                )
                nc.vector.tensor_tensor(
                    oh[:, 10:20, :], xb[:, 11:21, :], mb10, AluOpType.is_equal
                )
                nc.vector.tensor_tensor(
                    prod[:, :N16, :], oh[:, :N16, :], tb16[:], AluOpType.mult
                )
                nc.vector.tensor_tensor(
                    prod[:, N16:kdve, :], oh[:, N16:kdve, :],
                    tb8[:, : kdve - N16, :], AluOpType.mult,
                )
                nc.gpsimd.tensor_tensor(
                    prod[:, kdve:, :], oh[:, kdve:, :],
                    tb8[:, kdve - N16 :, :], AluOpType.mult,
                )

            # ---- ACT: gt sums for K_ACT classes ----
            for c in range(K_ACT):
                src = tb16[:, c, :] if c < N16 else tb8[:, c - N16, :]
                nc.scalar.activation(
                    junk32[:], src, Copy,
                    accum_out=gt_accs[img][:, j, c : c + 1],
                )

            saved[t] = dict(img=img, j=j, oh=oh, prod=prod, tb8=tb8, kdve=kdve)

            # ---- PE: contiguous delayed block ----
            if t >= 1:
                emit_pr_interA_gt(t - 1)
            if t >= 2:
                emit_interB(t - 2, stop_at_end=(saved[t - 2]["j"] == chunks - 1))
                del saved[t - 2]

            # img0 drain: its inter group stops at interB(3), emitted at t=5.
            if t == chunks + 1 and bpc > 1:
                drain_image(0)

        # Flush for the final chunk: pr tracks the iseq halves, then gt,
        # then interB(nsteps-2), then the final chunk's inter (DVE batches,
        # then its Pool classes carrying the group stop).
        st = saved[nsteps - 1]
        st6 = saved[nsteps - 2]
        img = st["img"]
        for c in range(10):
            nc.tensor.matmul(
                ps_pr[img][:, :], sel[:, c, :], st["oh"][:, c, :],
                start=False, stop=False,
            )
        for c in range(K_ACT, CT):
            nc.tensor.matmul(
                ps_gt[img][:, :], sel8[:, c, :], st["tb8"][:, c - N16, :],
                start=False, stop=(c == CT - 1),
            )
        for c in range(10, CT):
            nc.tensor.matmul(
                ps_pr[img][:, :], sel[:, c, :], st["oh"][:, c, :],
                start=False, stop=(c == CT - 1),
            )
        assert st6["img"] == img
        emit_interB(nsteps - 2, stop_at_end=False)
        for c in range(st["kdve"]):
            nc.tensor.matmul(
                ps_in[img][:, :], sel[:, c, :], st["prod"][:, c, :],
                start=False, stop=False,
            )
        emit_interB(nsteps - 1, stop_at_end=True)
        drain_image(bpc - 1)

    return nc


_BUILD_LOCK = threading.Lock()
_NC_CACHE = {}


def get_compiled_nc(key="full"):
    with _BUILD_LOCK:
        if key not in _NC_CACHE:
            nc = bacc.Bacc("TRN2", target_bir_lowering=False, debug=False)
            build_kernel_ir(nc)
            nc.compile()
            _NC_CACHE[key] = nc
        return _NC_CACHE[key]


def make_in_maps(input, target):
    """Full f32 inputs -> per-core input dicts (fp16 input, fp16+fp8 target)."""
    import ml_dtypes

    inp = np.asarray(input, dtype=np.float32)
    tgt = np.asarray(target, dtype=np.float32)
    assert inp.shape == (B, C, H, W), inp.shape
    inp16 = inp.astype(np.float16)
    # class 0 never scored; classes 1..N16 as fp16, the rest as fp8e4 (0/1
    # masks are exact in every 8-bit float format).
    tgt16 = np.ascontiguousarray(tgt[:, 1 : 1 + N16]).astype(np.float16)
    tgt8 = np.ascontiguousarray(tgt[:, 1 + N16 :]).astype(ml_dtypes.float8_e4m3)
    return [
        {
            "input": np.ascontiguousarray(inp16[i * BPC : (i + 1) * BPC]),
            "target": np.ascontiguousarray(tgt16[i * BPC : (i + 1) * BPC]),
            "target8": np.ascontiguousarray(tgt8[i * BPC : (i + 1) * BPC]),
        }
        for i in range(NCORES)
    ]


def combine_stats(stats_all, gtacc_all):
    """stats_all [B,CT,4], gtacc_all [B,P,chunks,K_ACT] -> (scores, counts)."""
    stats_all = np.asarray(stats_all, dtype=np.float64)
    pr_bc = stats_all[..., 0]
    inter_bc = stats_all[..., 1]
    gt_bc = stats_all[..., 2].copy()
    gt_act = np.asarray(gtacc_all, dtype=np.float64).sum(axis=(1, 2))  # [B,K_ACT]
    gt_bc[:, :K_ACT] = gt_act
    present = (gt_bc > 0).astype(np.float64)
    inter_s = (present * inter_bc).sum(0)
    union_s = (present * (gt_bc + pr_bc)).sum(0) - inter_s + 1e-7
    counts = present.sum(0)
    scores = (inter_s / union_s) * counts
    return scores.astype(np.float32), counts.astype(np.float32)


def kernel(input, target):
    nc = get_compiled_nc()
    in_maps = make_in_maps(input, target)
    res = run_bass_kernel_spmd(nc, in_maps, core_ids=list(range(NCORES)))
    stats_all = np.concatenate([r["stats"] for r in res.results], axis=0)
    gtacc_all = np.concatenate([r["gtacc"] for r in res.results], axis=0)
    return combine_stats(stats_all, gtacc_all)


if __name__ == "__main__":
    rng = np.random.default_rng(0)
    x = rng.standard_normal((B, C, H, W), dtype=np.float32)
    t = (rng.random((B, C, H, W)) < 0.05).astype(np.float32)
    s, c = kernel(input=x, target=t)
    print("scores:", s)
    print("counts:", c)
